# revision 16
# baseline (speedup 1.0000x reference)
"""HeterogeneousKANLayer forward on 8 Trainium2 NeuronCores.

Math (reference):
  xn    = tanh(x)                                  [B, I]
  base  = silu(xn)                                 [B, I]
  basis = exp(-((xn - c_j)/w)^2), c_j evenly spaced on [-1,1], w = 2/(C-1)
  out[b,o] = sum_{i,c} basis[b,i,c]*coef[i,o,c]*scale_sp[o,i]
           + sum_i base[b,i]*scale_base[o,i]

Kernel strategy (data-parallel over batch, 8 cores x 512 rows):
  One [512b, 5632k] @ [5632k, 512o] contraction per core; 10 Gaussian
  planes as fp8e4 DoubleRow matmuls.  Production schedule:
    DVE: t1 = -20.25*xn - 40.5 (ts 4x), sq0n = t1*xn (tt 2x)
         q6/q8 = a_d*xn + sq0n (stt), chains g_{j+1} = (e^db * r) * g_j
    ACT: tanh, Exp g0 = e^{sq0n+b}, r = e^{9xn}, Sq3+Exp3 anchor,
         Exp6/Exp8 (q anchors), th2 = tanh(xn/2) -- all one table set.
    GPS: xnth2 = xn*th2 (the only fast gpsimd op: tt fp16).
  Residual (scale_base rows identical): rank-1 v-path
    v[b] = sum_i 0.5*(xn + xn*th2)[i,b]*sbv[i]  via 8 thin matmuls,
    added into the psum banks with 4 outer-product (K=1) matmuls.
  Warm burst: PE HAM needs a fully-busy 3.4us window (worst-case 6.8us
  contiguous) to unthrottle 1.2->2.4GHz; gapless warm matmuls bridge
  from engine release (~6.1us) into the first real plane matmuls.
  Out: per-bank stop -> drain (ACT/DVE alternating) -> dma on 4 queues.
"""

import sys
import types

import numpy as np
import ml_dtypes

import concourse.bass as bass
import concourse.tile as tile
from concourse import bacc, mybir

N_CORES = 8
B = 4096
I = 512
O = 512
C = 10
BS = B // N_CORES          # batch rows per core (512)
NT = I // 128              # 4 i-tiles
W_SP = 2.0 / (C - 1)       # rbf width == center spacing (2/9)
IW2 = 1.0 / (W_SP * W_SP)  # 20.25
CENTERS = np.linspace(-1.0, 1.0, C)
A_J = 2.0 * CENTERS * IW2
B_J = -(CENTERS ** 2) * IW2
DB = B_J[1:] - B_J[:-1]              # 8,6,4,2,0,-2,-4,-6,-8
S_G = 64.0                           # fp8 plane scale
LN_S = float(np.log(S_G))

Q_ANCHORS = [6, 8]                   # DVE q + ACT Exp
CHAINS = [(1, 0), (2, 1), (4, 3), (5, 4), (7, 6), (9, 8)]
N_WARM_BULK = 32                     # N=256 warm MMs, 213ns cold each:
                                     # 6.8us contiguous guarantees a fully
                                     # busy HAM window -> 2.4GHz before the
                                     # dense matmul phase; post-flip gaps
                                     # <3.4us never re-throttle.
N_WARM_FINE = 2                      # small cushion before g0

_CACHE = {}


def _build(rank1):
    """Build and finalize the per-core Bass module (same on all cores)."""
    nc = bacc.Bacc("TRN2", target_bir_lowering=False, debug=False,
                   num_devices=N_CORES)
    f32 = mybir.dt.float32
    f16 = mybir.dt.float16
    fp8 = mybir.dt.float8e4
    DR = mybir.MatmulPerfMode.DoubleRow
    MUL = mybir.AluOpType.mult
    ADD = mybir.AluOpType.add
    EXP = mybir.ActivationFunctionType.Exp
    TANH = mybir.ActivationFunctionType.Tanh
    SQUARE = mybir.ActivationFunctionType.Square

    xt_d = nc.dram_tensor("xt", (128, NT, BS), f16, kind="ExternalInput")
    wf_d = nc.dram_tensor("wf", (128, 4 * C, O), fp8, kind="ExternalInput")
    sbv_d = nc.dram_tensor("sbv", (128, NT), f16, kind="ExternalInput")
    if not rank1:
        ws_d = nc.dram_tensor("ws", (128, NT, O), f16, kind="ExternalInput")
    out_d = nc.dram_tensor("out", (BS, O), f32, kind="ExternalOutput")

    with tile.TileContext(nc) as tc:
        with (
            tc.tile_pool(name="big", bufs=1) as big,
            tc.tile_pool(name="wpool", bufs=1) as wpool,
            tc.tile_pool(name="psum", bufs=1, space="PSUM") as psum,
        ):
            xt_sb = big.tile([128, NT, BS], f16, tag="xt")
            xn = big.tile([128, NT, BS], f16, tag="xn")
            t1 = big.tile([128, NT, BS], f16, tag="t1")
            sq0n = big.tile([128, NT, BS], f16, tag="sq0n")
            sq3 = big.tile([128, NT, BS], f16, tag="sq3")
            r_t = big.tile([128, NT, BS], f16, tag="r")
            qs = {j: big.tile([128, NT, BS], f16, name=f"q{j}")
                  for j in Q_ANCHORS}
            uq = {j: big.tile([128, NT, BS], f16, name=f"u{j}")
                  for j in Q_ANCHORS}
            th2 = big.tile([128, NT, BS], f16, tag="th2")
            xnth2 = big.tile([128, NT, BS], f16, tag="xnth2")
            g = [big.tile([128, NT, BS], fp8, name=f"g{j}") for j in range(C)]
            wf_sb = wpool.tile([128, 4 * C, O], fp8, name="wf_sb")
            sbv_sb = wpool.tile([128, NT], f16, name="sbv_sb")
            if not rank1:
                ws_sb = wpool.tile([128, NT, O], f16, name="ws_sb")
            v16 = big.tile([1, BS], f16, tag="v16")
            ones_o = big.tile([1, O], f16, tag="ones_o")
            wmm_s = big.tile([128, 2, 128], fp8, tag="wmm_s")
            wmm_m = big.tile([128, 2, 256], fp8, tag="wmm_m")
            # bias columns: [0] = ln(S)-20.25 (g0 Exp), [1] = -c_3 (Sq3),
            # [2] = lnS (sq3 Exp), [3..] = B_J[j]+lnS for q-anchors
            bias_sb = big.tile([128, 3 + len(Q_ANCHORS)], f32, tag="bias")

            # ---- gpsimd memsets; warm-mm inputs pinned to highest priority
            # so the warm burst starts at engine release.
            with tc.high_priority():
                nc.gpsimd.memset(wmm_s[:], 0.0)
                nc.gpsimd.memset(wmm_m[:], 0.0)
            nc.gpsimd.memset(ones_o[:], 1.0)
            nc.gpsimd.memset(bias_sb[:, 0:1], LN_S - IW2)
            nc.gpsimd.memset(bias_sb[:, 1:2], float(-CENTERS[3]))
            nc.gpsimd.memset(bias_sb[:, 2:3], LN_S)
            for ai, j in enumerate(Q_ANCHORS):
                nc.gpsimd.memset(bias_sb[:, 3 + ai:4 + ai],
                                 float(B_J[j] + LN_S))

            # ---- DMAs (single SP queue; criticality-ordered) ----
            def hs(h):
                return slice(2 * h, 2 * h + 2)

            nc.sync.dma_start(out=xt_sb[:, hs(0), :], in_=xt_d[:, hs(0), :])
            nc.sync.dma_start(out=sbv_sb[:, :], in_=sbv_d[:, :])
            nc.sync.dma_start(out=xt_sb[:, hs(1), :], in_=xt_d[:, hs(1), :])
            nc.sync.dma_start(out=wf_sb[:, 0:4, :], in_=wf_d[:, 0:4, :])
            # wf in chunks: the completion semaphore of each dma_start gates
            # the matmuls that read it, so chunk boundaries must match
            # plane need-times (one big dma would stall g1+ until ~20us).
            for (k0, k1) in [(4, 12), (12, 20), (20, 28), (28, 40)]:
                nc.sync.dma_start(out=wf_sb[:, k0:k1, :],
                                  in_=wf_d[:, k0:k1, :])
            if not rank1:
                nc.sync.dma_start(out=ws_sb[:, :, :], in_=ws_d[:, :, :])

            # ---- PSUM ----
            ps = [psum.tile([128, O], f32, name=f"ps{bt}") for bt in range(4)]
            warm_ps = psum.tile([128, O], f32, name="warm_ps")
            v_ps = psum.tile([1, BS], f32, name="v_ps")

            # ---- PE warm burst: gapless bridge into real matmuls ----
            for _ in range(N_WARM_BULK):
                nc.tensor.matmul(warm_ps[:, 0:256], wmm_s[:, :, :],
                                 wmm_m[:, :, :],
                                 start=True, stop=True, perf_mode=DR)

            def warm_fine(n):
                for _ in range(n):
                    nc.tensor.matmul(warm_ps[:, 0:128], wmm_s[:, :, :],
                                     wmm_m[:, :, 0:128],
                                     start=True, stop=True, perf_mode=DR)

            # ---- production (program order == scheduler priority) ----
            def act(out, in_, func, h, **kw):
                nc.scalar.activation(out=out[:, hs(h), :],
                                     in_=in_[:, hs(h), :], func=func, **kw)

            def stt(out, in0, scalar, in1, h, op0=MUL, op1=MUL):
                nc.vector.scalar_tensor_tensor(
                    out=out[:, hs(h), :], in0=in0[:, hs(h), :],
                    scalar=scalar, in1=in1[:, hs(h), :], op0=op0, op1=op1)

            # Production, emitted in CAUSAL order (Tile derives the dep
            # graph from program order: a reader must follow its writer).
            # The per-engine projections of this listing are the intended
            # engine FIFO programs.
            def t1_sq0n(h):
                nc.vector.tensor_scalar(
                    out=t1[:, hs(h), :], in0=xn[:, hs(h), :],
                    scalar1=float(-IW2), scalar2=float(-2.0 * IW2),
                    op0=MUL, op1=ADD)
                nc.vector.tensor_mul(sq0n[:, hs(h), :], t1[:, hs(h), :],
                                     xn[:, hs(h), :])

            def q_plane(j, h):
                # ts (4x mode) + tt add (2x) beats one stt (1x) by ~200ns
                nc.vector.tensor_scalar_mul(
                    uq[j][:, hs(h), :], xn[:, hs(h), :],
                    float(A_J[j] - A_J[0]))
                nc.vector.tensor_add(qs[j][:, hs(h), :], uq[j][:, hs(h), :],
                                     sq0n[:, hs(h), :])

            def chain(dst, src, h):
                stt(g[dst], r_t, float(np.exp(DB[src])), g[src], h)

            act(xn, xt_sb, TANH, 0)                              # ACT
            act(xn, xt_sb, TANH, 1)                              # ACT
            t1_sq0n(0)                                           # DVE x2
            t1_sq0n(1)                                           # DVE x2
            act(g[0], sq0n, EXP, 0, bias=bias_sb[:, 0:1])        # ACT
            act(r_t, xn, EXP, 0, scale=float(2.0 / W_SP))        # ACT
            act(g[0], sq0n, EXP, 1, bias=bias_sb[:, 0:1])        # ACT
            act(r_t, xn, EXP, 1, scale=float(2.0 / W_SP))        # ACT
            q_plane(6, 0)                                        # DVE
            q_plane(8, 0)                                        # DVE
            chain(1, 0, 0)                                       # DVE
            q_plane(6, 1)                                        # DVE
            chain(1, 0, 1)                                       # DVE
            chain(2, 1, 0)                                       # DVE
            q_plane(8, 1)                                        # DVE
            chain(2, 1, 1)                                       # DVE
            act(sq3, xn, SQUARE, 0, bias=bias_sb[:, 1:2])        # ACT
            act(g[3], sq3, EXP, 0, scale=float(-IW2),
                bias=bias_sb[:, 2:3])                            # ACT
            act(th2, xn, TANH, 0, scale=0.5)                     # ACT
            act(sq3, xn, SQUARE, 1, bias=bias_sb[:, 1:2])        # ACT
            act(g[3], sq3, EXP, 1, scale=float(-IW2),
                bias=bias_sb[:, 2:3])                            # ACT
            act(th2, xn, TANH, 1, scale=0.5)                     # ACT
            chain(4, 3, 0)                                       # DVE
            chain(4, 3, 1)                                       # DVE
            for h in (0, 1):                                     # DVE x2
                nc.vector.tensor_mul(xnth2[:, hs(h), :], xn[:, hs(h), :],
                                     th2[:, hs(h), :])
            act(g[6], qs[6], EXP, 0, bias=bias_sb[:, 3:4])       # ACT
            act(g[6], qs[6], EXP, 1, bias=bias_sb[:, 3:4])       # ACT
            act(g[8], qs[8], EXP, 0, bias=bias_sb[:, 4:5])       # ACT
            act(g[8], qs[8], EXP, 1, bias=bias_sb[:, 4:5])       # ACT
            chain(5, 4, 0)                                       # DVE
            chain(5, 4, 1)                                       # DVE
            chain(7, 6, 0)                                       # DVE
            chain(7, 6, 1)                                       # DVE
            chain(9, 8, 0)                                       # DVE
            chain(9, 8, 1)                                       # DVE

            # ---- PE stream: warm bridge, thin v MMs, planes in order ----
            def mm_g(j, p, start=False, stop=False):
                for bt in range(4):
                    nc.tensor.matmul(
                        ps[bt],
                        g[j][:, 2 * p:2 * p + 2, bt * 128:(bt + 1) * 128],
                        wf_sb[:, 4 * j + 2 * p:4 * j + 2 * p + 2, :],
                        start=start, stop=stop, perf_mode=DR)

            def thin(plane, t, start=False, stop=False):
                nc.tensor.matmul(v_ps, sbv_sb[:, t:t + 1], plane[:, t, :],
                                 start=start, stop=stop)

            if rank1:
                thin(xn, 0, start=True)
                thin(xn, 1)
                thin(xn, 2)
                thin(xn, 3)
            warm_fine(N_WARM_FINE)
            def pads(n):
                # keep the HAM activity window busy across production-paced
                # idle gaps; at 2.4GHz each pad costs only ~107ns
                for _ in range(n):
                    nc.tensor.matmul(warm_ps[:, 0:256], wmm_s[:, :, :],
                                     wmm_m[:, :, :],
                                     start=True, stop=True, perf_mode=DR)

            mm_g(0, 0, start=True)
            pads(3)
            mm_g(0, 1)
            pads(3)
            for (j, p) in [(1, 0), (1, 1), (2, 0), (2, 1),
                           (3, 0), (3, 1), (4, 0), (4, 1),
                           (5, 0), (5, 1)]:
                mm_g(j, p)
                pads(3)
            for (j, p) in [(6, 0), (6, 1)]:
                mm_g(j, p)
            if rank1:
                for t in range(NT):
                    thin(xnth2, t, stop=(t == NT - 1))
                mm_g(7, 0)
                mm_g(7, 1)
                # v16 lands via ACT after Exp8 h1; outer adds v to banks
                nc.scalar.activation(out=v16[:, :], in_=v_ps[:, :],
                                     func=mybir.ActivationFunctionType.Copy,
                                     scale=float(S_G))
                mm_g(8, 0)
                for bt in range(4):
                    nc.tensor.matmul(
                        ps[bt], v16[0:1, bt * 128:(bt + 1) * 128],
                        ones_o[0:1, :], start=False, stop=False)
                mm_g(8, 1)
                mm_g(9, 0)
            else:
                for (j, p) in [(7, 0), (7, 1), (8, 0), (8, 1)]:
                    mm_g(j, p)
                for t in range(NT):
                    for bt in range(4):
                        nc.tensor.matmul(
                            ps[bt], xn[:, t, bt * 128:(bt + 1) * 128],
                            ws_sb[:, t, :], start=False, stop=False)
                mm_g(9, 0)
                for t in range(NT):
                    for bt in range(4):
                        nc.tensor.matmul(
                            ps[bt], xnth2[:, t, bt * 128:(bt + 1) * 128],
                            ws_sb[:, t, :], start=False, stop=False)
            # g9 pair-1 bank-major with stop; drains alternate ACT/DVE and
            # the four out-DMAs ride four different hardware queues.
            inv_s = float(1.0 / S_G)
            dma_engines = [nc.sync, nc.scalar, nc.sync, nc.scalar]
            for bt in range(4):
                nc.tensor.matmul(
                    ps[bt], g[9][:, 2:4, bt * 128:(bt + 1) * 128],
                    wf_sb[:, 38:40, :], start=False, stop=True, perf_mode=DR)
                o_sb = big.tile([128, O], f32, name=f"o{bt}")
                if bt % 2 == 0:
                    nc.scalar.mul(out=o_sb[:], in_=ps[bt][:], mul=inv_s)
                else:
                    nc.vector.tensor_scalar_mul(out=o_sb[:], in0=ps[bt][:],
                                                scalar1=inv_s)
                dma_engines[bt].dma_start(
                    out=out_d[bt * 128:(bt + 1) * 128, :], in_=o_sb[:])
            import os
            if os.environ.get("KAN_DEBUG_TAPS") == "1":
                taps = {"xn": xn, "sq0n": sq0n, "q6": qs[6], "g0": g[0],
                        "g6": g[6], "g9": g[9], "xnth2": xnth2, "th2": th2}
                for nm, t_sb in taps.items():
                    d_out = nc.dram_tensor(f"dbg_{nm}", (128, NT, BS),
                                           t_sb.dtype, kind="ExternalOutput")
                    nc.sync.dma_start(out=d_out[:, :, :], in_=t_sb[:, :, :])
                dv = nc.dram_tensor("dbg_v16", (1, BS), mybir.dt.float16,
                                    kind="ExternalOutput")
                nc.sync.dma_start(out=dv[:, :], in_=v16[:, :])
                db = nc.dram_tensor("dbg_bias", (128, 3 + len(Q_ANCHORS)),
                                    f32, kind="ExternalOutput")
                nc.sync.dma_start(out=db[:, :], in_=bias_sb[:, :])
    nc.finalize()
    return nc


def _prep_inputs(x, coef, scale_base, scale_sp):
    """Host-side shard + layout prep (cheap numpy reshapes/casts)."""
    x = np.asarray(x, dtype=np.float32)
    coef = np.asarray(coef, dtype=np.float32)
    scale_base = np.asarray(scale_base, dtype=np.float32)
    scale_sp = np.asarray(scale_sp, dtype=np.float32)

    # wf[p, kt, o] (partition-major for contiguous DMA), kt = 4*j + t.
    wfull = coef * scale_sp.T[:, :, None]                    # [I, O, C]
    wfull = wfull.reshape(NT, 128, O, C).transpose(3, 0, 1, 2)  # [C,NT,128,O]
    wf = np.clip(wfull.reshape(4 * C, 128, O), -240.0, 240.0).astype(
        ml_dtypes.float8_e4m3).transpose(1, 0, 2)            # [128, 4C, O]
    wf = np.ascontiguousarray(wf)
    # residual rank-1 when all scale_base rows are identical (the
    # reference ships ones); otherwise dense fallback on (xn, xnth2).
    rank1 = bool(np.all(scale_base == scale_base[0:1, :]))
    # v thin matmuls accumulate 0.5*(xn + xnth2) @ sbv; psum carries S_G
    # via the v16 copy scale, so sbv is just 0.5*scale_base[0].
    sbv = np.clip(0.5 * scale_base[0, :], -60000.0, 60000.0).astype(
        np.float16).reshape(NT, 128).T                       # [128, NT]
    sbv = np.ascontiguousarray(sbv)
    if not rank1:
        ws = np.ascontiguousarray(
            np.clip(0.5 * S_G * scale_base.T.reshape(NT, 128, O), -60000.0,
                    60000.0).transpose(1, 0, 2)).astype(np.float16)

    in_maps = []
    for k in range(N_CORES):
        xs = np.clip(x[k * BS:(k + 1) * BS, :], -30.0, 30.0)  # [BS, I]
        xt = np.ascontiguousarray(
            xs.T.reshape(NT, 128, BS).transpose(1, 0, 2)).astype(np.float16)
        m = {"xt": xt, "wf": wf, "sbv": sbv}
        if not rank1:
            m["ws"] = ws
        in_maps.append(m)
    return in_maps, rank1


def _run(in_maps, rank1, trace=False):
    if "antenv.axon_hooks" not in sys.modules:
        try:
            from trn_agent_boot.trn_boot import _ntff_profile_via_ctypes
            _hook = _ntff_profile_via_ctypes("/opt/axon/libaxon_pjrt.so")
            _mod = types.ModuleType("antenv.axon_hooks")
            _mod.get_axon_ntff_profile_hook = lambda: _hook
            sys.modules["antenv.axon_hooks"] = _mod
        except Exception:
            pass
    from concourse.bass_utils import run_bass_kernel_spmd

    key = ("nc", rank1)
    if key not in _CACHE:
        _CACHE[key] = _build(rank1)
    return run_bass_kernel_spmd(_CACHE[key], in_maps,
                                core_ids=list(range(N_CORES)), trace=trace)


def kernel(x, coef, scale_base, scale_sp):
    in_maps, rank1 = _prep_inputs(x, coef, scale_base, scale_sp)
    res = _run(in_maps, rank1, trace=False)
    out = np.concatenate([res.results[k]["out"] for k in range(N_CORES)],
                         axis=0)
    return out.astype(np.float32)


# revision 17
# speedup vs baseline: 1.1560x; 1.1560x over previous
"""HeterogeneousKANLayer forward on 8 Trainium2 NeuronCores.

Math (reference):
  xn    = tanh(x)                                  [B, I]
  base  = silu(xn)                                 [B, I]
  basis = exp(-((xn - c_j)/w)^2), c_j evenly spaced on [-1,1], w = 2/(C-1)
  out[b,o] = sum_{i,c} basis[b,i,c]*coef[i,o,c]*scale_sp[o,i]
           + sum_i base[b,i]*scale_base[o,i]

Kernel strategy (data-parallel over batch, 8 cores x 512 rows):
  One [512b, 5632k] @ [5632k, 512o] contraction per core; 10 Gaussian
  planes as fp8e4 DoubleRow matmuls.  Production schedule:
    DVE: t1 = -20.25*xn - 40.5 (ts 4x), sq0n = t1*xn (tt 2x)
         q6/q8 = a_d*xn + sq0n (stt), chains g_{j+1} = (e^db * r) * g_j
    ACT: tanh, Exp g0 = e^{sq0n+b}, r = e^{9xn}, Sq3+Exp3 anchor,
         Exp6/Exp8 (q anchors), th2 = tanh(xn/2) -- all one table set.
    GPS: xnth2 = xn*th2 (the only fast gpsimd op: tt fp16).
  Residual (scale_base rows identical): rank-1 v-path
    v[b] = sum_i 0.5*(xn + xn*th2)[i,b]*sbv[i]  via 8 thin matmuls,
    added into the psum banks with 4 outer-product (K=1) matmuls.
  Warm burst: PE HAM needs a fully-busy 3.4us window (worst-case 6.8us
  contiguous) to unthrottle 1.2->2.4GHz; gapless warm matmuls bridge
  from engine release (~6.1us) into the first real plane matmuls.
  Out: per-bank stop -> drain (ACT/DVE alternating) -> dma on 4 queues.
"""

import sys
import types

import numpy as np
import ml_dtypes

import concourse.bass as bass
import concourse.tile as tile
from concourse import bacc, mybir

N_CORES = 8
B = 4096
I = 512
O = 512
C = 10
BS = B // N_CORES          # batch rows per core (512)
NT = I // 128              # 4 i-tiles
W_SP = 2.0 / (C - 1)       # rbf width == center spacing (2/9)
IW2 = 1.0 / (W_SP * W_SP)  # 20.25
CENTERS = np.linspace(-1.0, 1.0, C)
A_J = 2.0 * CENTERS * IW2
B_J = -(CENTERS ** 2) * IW2
DB = B_J[1:] - B_J[:-1]              # 8,6,4,2,0,-2,-4,-6,-8
S_G = 64.0                           # fp8 plane scale
LN_S = float(np.log(S_G))

Q_ANCHORS = [6, 8]                   # DVE q + ACT Exp
CHAINS = [(1, 0), (2, 1), (4, 3), (5, 4), (7, 6), (9, 8)]
N_WARM_BULK = 32                     # N=256 warm MMs, 213ns cold each:
                                     # 6.8us contiguous guarantees a fully
                                     # busy HAM window -> 2.4GHz before the
                                     # dense matmul phase; post-flip gaps
                                     # <3.4us never re-throttle.
N_WARM_FINE = 2                      # small cushion before g0

_CACHE = {}


def _build(rank1):
    """Build and finalize the per-core Bass module (same on all cores)."""
    nc = bacc.Bacc("TRN2", target_bir_lowering=False, debug=False,
                   num_devices=N_CORES)
    f32 = mybir.dt.float32
    f16 = mybir.dt.float16
    fp8 = mybir.dt.float8e4
    DR = mybir.MatmulPerfMode.DoubleRow
    MUL = mybir.AluOpType.mult
    ADD = mybir.AluOpType.add
    EXP = mybir.ActivationFunctionType.Exp
    TANH = mybir.ActivationFunctionType.Tanh
    SQUARE = mybir.ActivationFunctionType.Square

    xt_d = nc.dram_tensor("xt", (128, NT, BS), f16, kind="ExternalInput")
    wf_d = nc.dram_tensor("wf", (128, 4 * C, O), fp8, kind="ExternalInput")
    sbv_d = nc.dram_tensor("sbv", (128, NT), f16, kind="ExternalInput")
    if not rank1:
        ws_d = nc.dram_tensor("ws", (128, NT, O), f16, kind="ExternalInput")
    out_d = nc.dram_tensor("out", (BS, O), f32, kind="ExternalOutput")

    with tile.TileContext(nc) as tc:
        with (
            tc.tile_pool(name="big", bufs=1) as big,
            tc.tile_pool(name="wpool", bufs=1) as wpool,
            tc.tile_pool(name="psum", bufs=1, space="PSUM") as psum,
        ):
            xt_sb = big.tile([128, NT, BS], f16, tag="xt")
            xn = big.tile([128, NT, BS], f16, tag="xn")
            t1 = big.tile([128, NT, BS], f16, tag="t1")
            sq0n = big.tile([128, NT, BS], f16, tag="sq0n")
            sq3 = big.tile([128, NT, BS], f16, tag="sq3")
            r_t = big.tile([128, NT, BS], f16, tag="r")
            qs = {j: big.tile([128, NT, BS], f16, name=f"q{j}")
                  for j in Q_ANCHORS}
            uq = {j: big.tile([128, NT, BS], f16, name=f"u{j}")
                  for j in Q_ANCHORS}
            th2 = big.tile([128, NT, BS], f16, tag="th2")
            xnth2 = big.tile([128, NT, BS], f16, tag="xnth2")
            g = [big.tile([128, NT, BS], fp8, name=f"g{j}") for j in range(C)]
            wf_sb = wpool.tile([128, 4 * C, O], fp8, name="wf_sb")
            sbv_sb = wpool.tile([128, NT], f16, name="sbv_sb")
            if not rank1:
                ws_sb = wpool.tile([128, NT, O], f16, name="ws_sb")
            v16 = big.tile([1, BS], f16, tag="v16")
            ones_o = big.tile([1, O], f16, tag="ones_o")
            wmm_s = big.tile([128, 2, 128], fp8, tag="wmm_s")
            wmm_m = big.tile([128, 2, 256], fp8, tag="wmm_m")
            # bias columns: [0] = ln(S)-20.25 (g0 Exp), [1] = -c_3 (Sq3),
            # [2] = lnS (sq3 Exp), [3..] = B_J[j]+lnS for q-anchors
            bias_sb = big.tile([128, 3 + len(Q_ANCHORS)], f32, tag="bias")

            # ---- gpsimd memsets; warm-mm inputs pinned to highest priority
            # so the warm burst starts at engine release.
            with tc.high_priority():
                nc.gpsimd.memset(wmm_s[:], 0.0)
                nc.gpsimd.memset(wmm_m[:], 0.0)
            nc.gpsimd.memset(ones_o[:], 1.0)
            nc.gpsimd.memset(bias_sb[:, 0:1], LN_S - IW2)
            nc.gpsimd.memset(bias_sb[:, 1:2], float(-CENTERS[3]))
            nc.gpsimd.memset(bias_sb[:, 2:3], LN_S)
            for ai, j in enumerate(Q_ANCHORS):
                nc.gpsimd.memset(bias_sb[:, 3 + ai:4 + ai],
                                 float(B_J[j] + LN_S))

            # ---- DMAs (single SP queue; criticality-ordered) ----
            def hs(h):
                return slice(2 * h, 2 * h + 2)

            nc.sync.dma_start(out=xt_sb[:, hs(0), :], in_=xt_d[:, hs(0), :])
            nc.sync.dma_start(out=sbv_sb[:, :], in_=sbv_d[:, :])
            nc.sync.dma_start(out=xt_sb[:, hs(1), :], in_=xt_d[:, hs(1), :])
            nc.sync.dma_start(out=wf_sb[:, 0:4, :], in_=wf_d[:, 0:4, :])
            # wf in chunks: the completion semaphore of each dma_start gates
            # the matmuls that read it, so chunk boundaries must match
            # plane need-times (one big dma would stall g1+ until ~20us).
            for (k0, k1) in [(4, 12), (12, 20), (20, 28), (28, 40)]:
                nc.sync.dma_start(out=wf_sb[:, k0:k1, :],
                                  in_=wf_d[:, k0:k1, :])
            if not rank1:
                nc.sync.dma_start(out=ws_sb[:, :, :], in_=ws_d[:, :, :])

            # ---- PSUM ----
            ps = [psum.tile([128, O], f32, name=f"ps{bt}") for bt in range(4)]
            warm_ps = psum.tile([128, O], f32, name="warm_ps")
            v_ps = psum.tile([1, BS], f32, name="v_ps")

            # ---- PE warm burst: gapless bridge into real matmuls ----
            for _ in range(N_WARM_BULK):
                nc.tensor.matmul(warm_ps[:, 0:256], wmm_s[:, :, :],
                                 wmm_m[:, :, :],
                                 start=True, stop=True, perf_mode=DR)

            def warm_fine(n):
                for _ in range(n):
                    nc.tensor.matmul(warm_ps[:, 0:128], wmm_s[:, :, :],
                                     wmm_m[:, :, 0:128],
                                     start=True, stop=True, perf_mode=DR)

            # ---- production (program order == scheduler priority) ----
            def act(out, in_, func, h, **kw):
                nc.scalar.activation(out=out[:, hs(h), :],
                                     in_=in_[:, hs(h), :], func=func, **kw)

            def stt(out, in0, scalar, in1, h, op0=MUL, op1=MUL):
                nc.vector.scalar_tensor_tensor(
                    out=out[:, hs(h), :], in0=in0[:, hs(h), :],
                    scalar=scalar, in1=in1[:, hs(h), :], op0=op0, op1=op1)

            # Production, emitted in CAUSAL order (Tile derives the dep
            # graph from program order: a reader must follow its writer).
            # The per-engine projections of this listing are the intended
            # engine FIFO programs.
            def t1_sq0n(h):
                nc.vector.tensor_scalar(
                    out=t1[:, hs(h), :], in0=xn[:, hs(h), :],
                    scalar1=float(-IW2), scalar2=float(-2.0 * IW2),
                    op0=MUL, op1=ADD)
                nc.vector.tensor_mul(sq0n[:, hs(h), :], t1[:, hs(h), :],
                                     xn[:, hs(h), :])

            def q_plane(j, h):
                # ts (4x mode) + tt add (2x) beats one stt (1x) by ~200ns
                nc.vector.tensor_scalar_mul(
                    uq[j][:, hs(h), :], xn[:, hs(h), :],
                    float(A_J[j] - A_J[0]))
                nc.vector.tensor_add(qs[j][:, hs(h), :], uq[j][:, hs(h), :],
                                     sq0n[:, hs(h), :])

            def chain(dst, src, h):
                stt(g[dst], r_t, float(np.exp(DB[src])), g[src], h)

            act(xn, xt_sb, TANH, 0)                              # ACT
            act(xn, xt_sb, TANH, 1)                              # ACT
            t1_sq0n(0)                                           # DVE x2
            t1_sq0n(1)                                           # DVE x2
            act(g[0], sq0n, EXP, 0, bias=bias_sb[:, 0:1])        # ACT
            act(r_t, xn, EXP, 0, scale=float(2.0 / W_SP))        # ACT
            act(g[0], sq0n, EXP, 1, bias=bias_sb[:, 0:1])        # ACT
            act(r_t, xn, EXP, 1, scale=float(2.0 / W_SP))        # ACT
            q_plane(6, 0)                                        # DVE
            q_plane(8, 0)                                        # DVE
            chain(1, 0, 0)                                       # DVE
            q_plane(6, 1)                                        # DVE
            chain(1, 0, 1)                                       # DVE
            chain(2, 1, 0)                                       # DVE
            q_plane(8, 1)                                        # DVE
            chain(2, 1, 1)                                       # DVE
            act(sq3, xn, SQUARE, 0, bias=bias_sb[:, 1:2])        # ACT
            act(g[3], sq3, EXP, 0, scale=float(-IW2),
                bias=bias_sb[:, 2:3])                            # ACT
            act(th2, xn, TANH, 0, scale=0.5)                     # ACT
            act(sq3, xn, SQUARE, 1, bias=bias_sb[:, 1:2])        # ACT
            act(g[3], sq3, EXP, 1, scale=float(-IW2),
                bias=bias_sb[:, 2:3])                            # ACT
            act(th2, xn, TANH, 1, scale=0.5)                     # ACT
            chain(4, 3, 0)                                       # DVE
            chain(4, 3, 1)                                       # DVE
            for h in (0, 1):                                     # DVE x2
                nc.vector.tensor_mul(xnth2[:, hs(h), :], xn[:, hs(h), :],
                                     th2[:, hs(h), :])
            act(g[6], qs[6], EXP, 0, bias=bias_sb[:, 3:4])       # ACT
            act(g[6], qs[6], EXP, 1, bias=bias_sb[:, 3:4])       # ACT
            act(g[8], qs[8], EXP, 0, bias=bias_sb[:, 4:5])       # ACT
            act(g[8], qs[8], EXP, 1, bias=bias_sb[:, 4:5])       # ACT
            chain(5, 4, 0)                                       # DVE
            chain(5, 4, 1)                                       # DVE
            chain(7, 6, 0)                                       # DVE
            chain(7, 6, 1)                                       # DVE
            chain(9, 8, 0)                                       # DVE
            chain(9, 8, 1)                                       # DVE

            # ---- PE stream: warm bridge, thin v MMs, planes in order ----
            def mm_g(j, p, start=False, stop=False):
                for bt in range(4):
                    nc.tensor.matmul(
                        ps[bt],
                        g[j][:, 2 * p:2 * p + 2, bt * 128:(bt + 1) * 128],
                        wf_sb[:, 4 * j + 2 * p:4 * j + 2 * p + 2, :],
                        start=start, stop=stop, perf_mode=DR)

            def thin(plane, t, start=False, stop=False):
                nc.tensor.matmul(v_ps, sbv_sb[:, t:t + 1], plane[:, t, :],
                                 start=start, stop=stop)

            if rank1:
                thin(xn, 0, start=True)
                thin(xn, 1)
                thin(xn, 2)
                thin(xn, 3)
            warm_fine(N_WARM_FINE)
            mm_g(0, 0, start=True)
            mm_g(0, 1)
            for (j, p) in [(1, 0), (1, 1), (2, 0), (2, 1),
                           (3, 0), (3, 1), (4, 0), (4, 1),
                           (5, 0), (5, 1), (6, 0), (6, 1)]:
                mm_g(j, p)
            if rank1:
                for t in range(NT):
                    thin(xnth2, t, stop=(t == NT - 1))
                mm_g(7, 0)
                mm_g(7, 1)
                # v16 lands via ACT after Exp8 h1; outer adds v to banks
                nc.scalar.activation(out=v16[:, :], in_=v_ps[:, :],
                                     func=mybir.ActivationFunctionType.Copy,
                                     scale=float(S_G))
                mm_g(8, 0)
                for bt in range(4):
                    nc.tensor.matmul(
                        ps[bt], v16[0:1, bt * 128:(bt + 1) * 128],
                        ones_o[0:1, :], start=False, stop=False)
                mm_g(8, 1)
                mm_g(9, 0)
            else:
                for (j, p) in [(7, 0), (7, 1), (8, 0), (8, 1)]:
                    mm_g(j, p)
                for t in range(NT):
                    for bt in range(4):
                        nc.tensor.matmul(
                            ps[bt], xn[:, t, bt * 128:(bt + 1) * 128],
                            ws_sb[:, t, :], start=False, stop=False)
                mm_g(9, 0)
                for t in range(NT):
                    for bt in range(4):
                        nc.tensor.matmul(
                            ps[bt], xnth2[:, t, bt * 128:(bt + 1) * 128],
                            ws_sb[:, t, :], start=False, stop=False)
            # g9 pair-1 bank-major with stop; drains alternate ACT/DVE and
            # the four out-DMAs ride four different hardware queues.
            inv_s = float(1.0 / S_G)
            dma_engines = [nc.sync, nc.scalar, nc.sync, nc.scalar]
            for bt in range(4):
                nc.tensor.matmul(
                    ps[bt], g[9][:, 2:4, bt * 128:(bt + 1) * 128],
                    wf_sb[:, 38:40, :], start=False, stop=True, perf_mode=DR)
                o_sb = big.tile([128, O], f32, name=f"o{bt}")
                if bt % 2 == 0:
                    nc.scalar.mul(out=o_sb[:], in_=ps[bt][:], mul=inv_s)
                else:
                    nc.vector.tensor_scalar_mul(out=o_sb[:], in0=ps[bt][:],
                                                scalar1=inv_s)
                dma_engines[bt].dma_start(
                    out=out_d[bt * 128:(bt + 1) * 128, :], in_=o_sb[:])
            import os
            if os.environ.get("KAN_DEBUG_TAPS") == "1":
                taps = {"xn": xn, "sq0n": sq0n, "q6": qs[6], "g0": g[0],
                        "g6": g[6], "g9": g[9], "xnth2": xnth2, "th2": th2}
                for nm, t_sb in taps.items():
                    d_out = nc.dram_tensor(f"dbg_{nm}", (128, NT, BS),
                                           t_sb.dtype, kind="ExternalOutput")
                    nc.sync.dma_start(out=d_out[:, :, :], in_=t_sb[:, :, :])
                dv = nc.dram_tensor("dbg_v16", (1, BS), mybir.dt.float16,
                                    kind="ExternalOutput")
                nc.sync.dma_start(out=dv[:, :], in_=v16[:, :])
                db = nc.dram_tensor("dbg_bias", (128, 3 + len(Q_ANCHORS)),
                                    f32, kind="ExternalOutput")
                nc.sync.dma_start(out=db[:, :], in_=bias_sb[:, :])
    nc.finalize()
    return nc


def _prep_inputs(x, coef, scale_base, scale_sp):
    """Host-side shard + layout prep (cheap numpy reshapes/casts)."""
    x = np.asarray(x, dtype=np.float32)
    coef = np.asarray(coef, dtype=np.float32)
    scale_base = np.asarray(scale_base, dtype=np.float32)
    scale_sp = np.asarray(scale_sp, dtype=np.float32)

    # wf[p, kt, o] (partition-major for contiguous DMA), kt = 4*j + t.
    wfull = coef * scale_sp.T[:, :, None]                    # [I, O, C]
    wfull = wfull.reshape(NT, 128, O, C).transpose(3, 0, 1, 2)  # [C,NT,128,O]
    wf = np.clip(wfull.reshape(4 * C, 128, O), -240.0, 240.0).astype(
        ml_dtypes.float8_e4m3).transpose(1, 0, 2)            # [128, 4C, O]
    wf = np.ascontiguousarray(wf)
    # residual rank-1 when all scale_base rows are identical (the
    # reference ships ones); otherwise dense fallback on (xn, xnth2).
    rank1 = bool(np.all(scale_base == scale_base[0:1, :]))
    # v thin matmuls accumulate 0.5*(xn + xnth2) @ sbv; psum carries S_G
    # via the v16 copy scale, so sbv is just 0.5*scale_base[0].
    sbv = np.clip(0.5 * scale_base[0, :], -60000.0, 60000.0).astype(
        np.float16).reshape(NT, 128).T                       # [128, NT]
    sbv = np.ascontiguousarray(sbv)
    if not rank1:
        ws = np.ascontiguousarray(
            np.clip(0.5 * S_G * scale_base.T.reshape(NT, 128, O), -60000.0,
                    60000.0).transpose(1, 0, 2)).astype(np.float16)

    in_maps = []
    for k in range(N_CORES):
        xs = np.clip(x[k * BS:(k + 1) * BS, :], -30.0, 30.0)  # [BS, I]
        xt = np.ascontiguousarray(
            xs.T.reshape(NT, 128, BS).transpose(1, 0, 2)).astype(np.float16)
        m = {"xt": xt, "wf": wf, "sbv": sbv}
        if not rank1:
            m["ws"] = ws
        in_maps.append(m)
    return in_maps, rank1


def _run(in_maps, rank1, trace=False):
    if "antenv.axon_hooks" not in sys.modules:
        try:
            from trn_agent_boot.trn_boot import _ntff_profile_via_ctypes
            _hook = _ntff_profile_via_ctypes("/opt/axon/libaxon_pjrt.so")
            _mod = types.ModuleType("antenv.axon_hooks")
            _mod.get_axon_ntff_profile_hook = lambda: _hook
            sys.modules["antenv.axon_hooks"] = _mod
        except Exception:
            pass
    from concourse.bass_utils import run_bass_kernel_spmd

    key = ("nc", rank1)
    if key not in _CACHE:
        _CACHE[key] = _build(rank1)
    return run_bass_kernel_spmd(_CACHE[key], in_maps,
                                core_ids=list(range(N_CORES)), trace=trace)


def kernel(x, coef, scale_base, scale_sp):
    in_maps, rank1 = _prep_inputs(x, coef, scale_base, scale_sp)
    res = _run(in_maps, rank1, trace=False)
    out = np.concatenate([res.results[k]["out"] for k in range(N_CORES)],
                         axis=0)
    return out.astype(np.float32)
